# revision 21
# baseline (speedup 1.0000x reference)
"""MoE FFN (8 experts, top-2) on 8 Trainium2 NeuronCores.

Hidden-dimension sharding for perfect load balance: every core processes ALL
16384 (token, expert) pairs, but only H/8 = 512 of each expert's 4096 hidden
units. Per-core work is exactly 2048 token-equivalents regardless of routing
skew. Each core holds H-slice weights of all 8 experts (16.8 MB bf16,
SBUF-resident) and emits a partial output in bf16; the host sums the 8
partials and applies the combine weights. b2 is fed as b2/8 so the partial
sum reproduces the bias exactly once.

The token stream is sorted by expert; tile boundaries are specialized to the
routing at build time (compile cache keyed on the per-expert counts), so tiles
never straddle an expert boundary and no capacity padding exists anywhere.

v2 schedule notes (from the v1 perfetto trace, 486.9us):
- HAM kept the PE at half clock for the first 36.5us: the 36x N=128 warmup
  burst was only ~4.2us of busy, and the HAM SHORT window (3.4us, free
  running) only fires on a FULLY busy window -> needs ~7us continuous busy
  to fire reliably. v2 burns 17 N=512 matmuls (~7.3us cold) on a memset
  tile; everything after runs at 2.4 GHz.
- The v1 ramp split x/w into 16 small per-ko DMAs; at ~650ns HWDGE issue
  cost each, the ramp was issue-bound, starving the PE until ~36us. v2
  host-packs x per tile ([P, KO, T] contiguous) and weights per expert
  ([P, KO*D] contiguous) so every transfer is one large DMA.
- v1 put x-prefetch + JIT weight DMAs on the Scalar queue AHEAD of the
  gelu ACTIVATEs in FIFO order; a weight DMA waiting on a semaphore lane
  parked at the queue head and delayed gelus -> 4.3us PE stall at 76us.
  v2 emits all input DMAs AFTER the tile's gelus.
- v1 issued 8 per-co output DMAs per tile (5.5us of Sync issue slots);
  v2 fuses them into 2 half-tile DMAs gated on the co3/co7 evictions
  (the last tile: 1 fused DMA for the shortest tail).

On-device layout: all matmul operands keep the contraction dim on SBUF
partitions. PSUM accumulates in f32; layer-1 bias rides the gelu on ScalarE,
layer-2 bias (pre-divided by 8) is fused into the PSUM eviction on VectorE,
which also downcasts the partial to bf16.
"""

import numpy as np
import ml_dtypes

N_EXPERTS = 8
TOP_K = 2
C = 1024
H = 4096
HS = H // N_EXPERTS      # per-core hidden slice
P = 128
T_TILE = 512
KO1 = C // P             # 8 contraction chunks for layer 1
KO2 = HS // P            # 4 contraction chunks for layer 2
MO1 = HS // P            # 4 output chunks for layer 1
CO2 = C // P             # 8 output chunks for layer 2
TP = 16384               # total (token, expert) pairs: 8192 tokens * top-2
WARMUP_MM = 18           # bridge the PE from the preamble (~3us) to first-x
                         # arrival (~12.5-13us with 0.25MB first pieces over 3
                         # DMA queues): cold warmup MM is ~534ns (LDW+stream at
                         # 1.2GHz), so 18 ends ~12.7us; any residual idle stays
                         # under the 3.4us HAM re-throttle window

_nc_cache = {}


def _tile_plan(counts):
    """Balanced single-expert tiles (<=512 wide) over the expert-sorted pair
    stream, plus a small final tile so the post-stream drain (evictions +
    output DMA of the last tile) is short."""
    tiles = []
    t0 = 0
    for e, c in enumerate(counts):
        if c == 0:
            continue
        k = -(-c // T_TILE)
        for i in range(k):
            T = c // k + (1 if i < c % k else 0)
            if T == 0:
                continue
            tiles.append((e, t0, T))
            t0 += T
    if tiles and tiles[-1][2] > 192:
        e, t0_, T = tiles.pop()
        tiles.append((e, t0_, T - 64))
        tiles.append((e, t0_ + T - 64, 64))
    assert sum(t for _, _, t in tiles) == sum(counts)
    return tiles


def _build_nc(counts):
    import concourse.mybir as mybir
    import concourse.tile as tile
    from concourse import bacc

    bf16 = mybir.dt.bfloat16
    f32 = mybir.dt.float32

    tiles = _tile_plan(counts)
    n_tiles = len(tiles)
    used = []
    for e, _, _ in tiles:
        if e not in used:
            used.append(e)

    nc = bacc.Bacc()
    # host-packed, fully contiguous per-tile / per-expert layouts.
    # weights are m-major ([P, m, ko, 128]) so a half-slab DMA delivers
    # complete (m, ko) weight tiles and compute can start off one piece.
    xt = nc.dram_tensor("xtp", [n_tiles * P, KO1 * T_TILE], bf16, kind="ExternalInput")
    w1 = {
        e: nc.dram_tensor(f"w1_{e}", [P, MO1 * KO1 * P], bf16, kind="ExternalInput")
        for e in used
    }
    w2 = {
        e: nc.dram_tensor(f"w2_{e}", [P, CO2 * KO2 * P], bf16, kind="ExternalInput")
        for e in used
    }
    # biases host-pre-swizzled and packed: [P, e*MO1+m] / [P, e*CO2+co]
    b1 = nc.dram_tensor("b1all", [P, N_EXPERTS * MO1], f32, kind="ExternalInput")
    b2 = nc.dram_tensor("b2all", [P, N_EXPERTS * CO2], f32, kind="ExternalInput")
    yt = nc.dram_tensor("ytp", [n_tiles * P, CO2 * T_TILE], bf16, kind="ExternalOutput")
    # the last tile gets its own fully-contiguous output region: the sliced
    # ytp write for a narrow tile degrades to 256B descriptor runs, which
    # cost ~3.4us on the kernel's serial tail
    T_last = tiles[-1][2]
    ytail = nc.dram_tensor("ytail", [P, CO2 * T_last], bf16, kind="ExternalOutput")

    xt_r = xt.rearrange("(n p) (k t) -> n p k t", p=P, t=T_TILE)
    w1_r = {e: w.rearrange("p (m k h) -> p m k h", m=MO1, k=KO1) for e, w in w1.items()}
    w2_r = {e: w.rearrange("p (c k h) -> p c k h", c=CO2, k=KO2) for e, w in w2.items()}
    yt_r = yt.rearrange("(n p) (c t) -> n p c t", p=P, t=T_TILE)
    ytail_r = ytail.rearrange("p (c t) -> p c t", c=CO2)

    gelu = mybir.ActivationFunctionType.Gelu_apprx_tanh

    with tile.TileContext(nc) as tc:
        with (
            tc.tile_pool(name="const", bufs=1) as const,
            tc.tile_pool(name="xp", bufs=4) as xp,
            tc.tile_pool(name="gp", bufs=2) as gp,
            tc.tile_pool(name="yp", bufs=2) as yp,
            tc.tile_pool(name="psum", bufs=8, space="PSUM") as psum,
        ):
            w1_sb = {
                e: const.tile([P, MO1, KO1, P], bf16, tag=f"w1_{e}", name=f"w1s{e}")
                for e in used
            }
            w2_sb = {
                e: const.tile([P, CO2, KO2, P], bf16, tag=f"w2_{e}", name=f"w2s{e}")
                for e in used
            }
            b1_sb = const.tile([P, N_EXPERTS * MO1], f32, tag="b1")
            b2_sb = const.tile([P, N_EXPERTS * CO2], f32, tag="b2")

            # HAM warmup: the PE cold-starts clock-gated at half rate; the
            # free-running 4096-cycle activity window needs ~2 windows of
            # continuous busy to fire reliably. ~7.3us of dummy N=512
            # matmuls on a memset tile while the first x/w DMAs stream in.
            warm = const.tile([P, T_TILE], bf16, tag="warm")
            nc.gpsimd.memset(warm[:], 0)
            pw = psum.tile([P, T_TILE], mybir.dt.float32, tag="ps", name="pwarm")
            for _ in range(WARMUP_MM):
                nc.tensor.matmul(pw[:, :], warm[:, :P], warm[:, :], start=True, stop=True)

            # Prologue: the 8-core simultaneous ramp runs at ~220 GB/s
            # aggregate, so order strictly by first-need across both HWDGE
            # queues and split the critical first slabs in half so compute
            # starts on ~0.5 MB pieces.
            e0 = used[0]
            x_tiles = {}
            for tj in range(min(3, n_tiles)):
                x_tiles[tj] = xp.tile([P, KO1, T_TILE], bf16, tag="x", name=f"x{tj}")
            # first-need pieces are 0.25MB so compute can start ~13us
            nc.sync.dma_start(x_tiles[0][:, 0:2, :], xt_r[0, :, 0:2, :])
            nc.scalar.dma_start(b1_sb[:], b1[:])
            nc.scalar.dma_start(b2_sb[:], b2[:])
            nc.scalar.dma_start(w1_sb[e0][:, 0:1, :, :], w1_r[e0][:, 0:1, :, :])
            nc.sync.dma_start(x_tiles[0][:, 2:4, :], xt_r[0, :, 2:4, :])
            nc.scalar.dma_start(w1_sb[e0][:, 1:2, :, :], w1_r[e0][:, 1:2, :, :])
            nc.sync.dma_start(x_tiles[0][:, 4:8, :], xt_r[0, :, 4:8, :])
            nc.scalar.dma_start(w1_sb[e0][:, 2:4, :, :], w1_r[e0][:, 2:4, :, :])
            nc.sync.dma_start(w2_sb[e0][:, 0:4, :, :], w2_r[e0][:, 0:4, :, :])
            nc.scalar.dma_start(w2_sb[e0][:, 4:8, :, :], w2_r[e0][:, 4:8, :, :])
            if n_tiles > 1:
                nc.sync.dma_start(x_tiles[1][:, 0:4, :], xt_r[1, :, 0:4, :])
                nc.scalar.dma_start(x_tiles[1][:, 4:8, :], xt_r[1, :, 4:8, :])
            if n_tiles > 2:
                nc.sync.dma_start(x_tiles[2][:, :, :], xt_r[2, :, :, :])
            if len(used) > 1:
                e1 = used[1]
                nc.sync.dma_start(w1_sb[e1][:, :, :, :], w1_r[e1][:, :, :, :])
                nc.scalar.dma_start(w2_sb[e1][:, :, :, :], w2_r[e1][:, :, :, :])

            # Just-in-time weight schedule: expert used[ui]'s two DMAs are
            # spread across the tiles of expert used[ui-1] (experts 0-1 load
            # in the prologue). Emitted AFTER the tile's gelus so a
            # sem-gated weight DMA can never park ahead of an ACTIVATE.
            tiles_of = {}
            for ti, (e, _, _) in enumerate(tiles):
                tiles_of.setdefault(e, []).append(ti)
            sched = {ti: [] for ti in range(n_tiles)}
            for ui in range(2, len(used)):
                e = used[ui]
                slots = tiles_of[used[ui - 1]]
                chunks = [
                    (w1_sb[e][:, :, :, :], w1_r[e][:, :, :, :]),
                    (w2_sb[e][:, :, :, :], w2_r[e][:, :, :, :]),
                ]
                for ci, ch in enumerate(chunks):
                    sched[slots[min(ci * len(slots) // len(chunks), len(slots) - 1)]].append(ch)

            for ti, (e, t0, T) in enumerate(tiles):
                x_sb = x_tiles.pop(ti)
                g_sb = gp.tile([P, KO2, T_TILE], bf16, tag="g")
                for m in range(MO1):
                    ph = psum.tile([P, T_TILE], mybir.dt.float32, tag="ps")
                    for ko in range(KO1):
                        nc.tensor.matmul(
                            ph[:, :T],
                            w1_sb[e][:, m, ko, :],
                            x_sb[:, ko, :T],
                            start=(ko == 0),
                            stop=(ko == KO1 - 1),
                        )
                    nc.scalar.activation(
                        g_sb[:, m, :T],
                        ph[:, :T],
                        gelu,
                        bias=b1_sb[:, e * MO1 + m : e * MO1 + m + 1],
                    )
                # ALL steady-state DMAs ride the Sync queue: the Scalar
                # queue carries only ACTIVATEs, so a semaphore-parked DMA
                # at a queue head can never delay a gelu (that head-of-line
                # blocking cost 2-4us PE stalls in v2/v3). Everything on
                # the Sync queue has at least a tile of slack.
                if ti + 3 < n_tiles:
                    ne, nt0, nt = tiles[ti + 3]
                    x_tiles[ti + 3] = xp.tile(
                        [P, KO1, T_TILE], bf16, tag="x", name=f"x{ti + 3}"
                    )
                    nc.sync.dma_start(
                        x_tiles[ti + 3][:, :, :], xt_r[ti + 3, :, :, :]
                    )
                for dst, src in sched[ti]:
                    nc.sync.dma_start(dst, src)

                y_sb = yp.tile([P, CO2, T_TILE], bf16, tag="y")
                for co in range(CO2):
                    py = psum.tile([P, T_TILE], mybir.dt.float32, tag="ps")
                    for ho in range(KO2):
                        nc.tensor.matmul(
                            py[:, :T],
                            w2_sb[e][:, co, ho, :],
                            g_sb[:, ho, :T],
                            start=(ho == 0),
                            stop=(ho == KO2 - 1),
                        )
                    # evictions alternate VectorE / ScalarE: halves the
                    # eviction latency chain (PSUM banks free sooner, and
                    # the last tile's eviction trail shortens)
                    if co % 2 == 0:
                        nc.vector.tensor_scalar_add(
                            y_sb[:, co, :T],
                            py[:, :T],
                            b2_sb[:, e * CO2 + co : e * CO2 + co + 1],
                        )
                    else:
                        nc.scalar.activation(
                            y_sb[:, co, :T],
                            py[:, :T],
                            mybir.ActivationFunctionType.Identity,
                            bias=b2_sb[:, e * CO2 + co : e * CO2 + co + 1],
                        )
                    # two fused half-tile output DMAs drain during compute
                    if ti + 1 < n_tiles:
                        if co == 3:
                            nc.sync.dma_start(
                                yt_r[ti, :, 0:4, :], y_sb[:, 0:4, :]
                            )
                        elif co == 7:
                            nc.sync.dma_start(
                                yt_r[ti, :, 4:8, :], y_sb[:, 4:8, :]
                            )
                    else:
                        # last tile: contiguous ytail region, two halves so
                        # the first half streams during the co4-7 chains
                        if co == 3:
                            nc.sync.dma_start(ytail_r[:, 0:4, :], y_sb[:, 0:4, :T])
                        elif co == 7:
                            nc.sync.dma_start(ytail_r[:, 4:8, :], y_sb[:, 4:8, :T])
    nc.finalize()
    return nc


def _route(flat_f32: np.ndarray, gate_w: np.ndarray):
    """Router, bit-matching the reference's jax ops (same env/backend)."""
    import jax
    import jax.numpy as jnp

    logits = jnp.asarray(flat_f32) @ jnp.asarray(gate_w).T
    probs = jax.nn.softmax(logits, axis=-1)
    top_p, top_i = jax.lax.top_k(probs, TOP_K)
    weights = top_p / (jnp.sum(top_p, axis=-1, keepdims=True) + 1e-8)
    return np.asarray(top_i), np.asarray(weights)


# results of the last device run, for test harness introspection
last_result = None


def _ensure_ntff_hook():
    """bass_utils' trace path imports antenv.axon_hooks, which the agent
    image's antenv lacks. Build the hook from trn_agent_boot's ctypes
    shim and inject a stand-in module."""
    import sys
    import types

    if "antenv.axon_hooks" in sys.modules:
        return
    try:
        from trn_agent_boot.trn_boot import _ntff_profile_via_ctypes

        hook = _ntff_profile_via_ctypes("/opt/axon/libaxon_pjrt.so")
    except Exception:
        hook = None
    m = types.ModuleType("antenv.axon_hooks")
    m.get_axon_ntff_profile_hook = lambda: hook
    m.set_axon_ntff_profile_hook = lambda h: None
    sys.modules["antenv.axon_hooks"] = m


def kernel(x, gate_w, w1, b1, w2, b2):
    from concourse.bass_utils import run_bass_kernel_spmd

    x = np.asarray(x)
    B, N, _ = x.shape
    flat = np.ascontiguousarray(x.reshape(-1, C), dtype=np.float32)
    T = flat.shape[0]
    assert T * TOP_K == TP

    top_i, weights = _route(flat, np.asarray(gate_w, dtype=np.float32))

    # expert-sorted pair stream
    tok_e = []
    wgt_e = []
    for e in range(N_EXPERTS):
        rows, cols = np.nonzero(top_i == e)
        tok_e.append(rows.astype(np.int64))
        wgt_e.append(weights[rows, cols].astype(np.float32))
    counts = tuple(len(i) for i in tok_e)
    pair_tok = np.concatenate(tok_e)
    pair_w = np.concatenate(wgt_e)

    nc = _nc_cache.get(counts)
    if nc is None:
        nc = _build_nc(counts)
        _nc_cache[counts] = nc

    tiles = _tile_plan(counts)
    n_tiles = len(tiles)

    bf16 = ml_dtypes.bfloat16
    xs = np.ascontiguousarray(flat[pair_tok].T).astype(bf16)  # [C, TP]
    # pack x per tile: [n_tiles*P, KO1*T_TILE], tile ti row p holds
    # [ko, t] -> xs[ko*P + p, t0+t]
    xs3 = xs.reshape(KO1, P, TP)
    xtp = np.zeros((n_tiles, P, KO1, T_TILE), dtype=bf16)
    for ti, (e, t0, Tt) in enumerate(tiles):
        xtp[ti, :, :, :Tt] = xs3[:, :, t0 : t0 + Tt].transpose(1, 0, 2)
    xtp = xtp.reshape(n_tiles * P, KO1 * T_TILE)

    w1 = np.asarray(w1, dtype=np.float32)
    w2 = np.asarray(w2, dtype=np.float32)
    b1 = np.asarray(b1, dtype=np.float32)
    b2 = np.asarray(b2, dtype=np.float32)

    in_maps = []
    for k in range(N_EXPERTS):
        hs = slice(k * HS, (k + 1) * HS)
        b1k = np.ascontiguousarray(
            b1[:, hs].reshape(N_EXPERTS, MO1, P).transpose(2, 0, 1).reshape(P, -1)
        )
        b2k = np.ascontiguousarray(
            (b2 / N_EXPERTS).reshape(N_EXPERTS, CO2, P).transpose(2, 0, 1).reshape(P, -1)
        )
        im = {"xtp": xtp, "b1all": b1k, "b2all": b2k}
        for e in range(N_EXPERTS):
            # w1 lhsT m-major: [ki, m, ko, hm] = w1[e, hs0 + m*P+hm, ko*P+ki]
            a = w1[e, hs, :].reshape(MO1, P, KO1, P).transpose(3, 0, 2, 1)
            im[f"w1_{e}"] = np.ascontiguousarray(a.reshape(P, MO1 * KO1 * P)).astype(bf16)
            # w2 lhsT co-major: [ki, co, ho, cm] = w2[e, co*P+cm, hs0 + ho*P+ki]
            b = w2[e, :, hs].reshape(CO2, P, KO2, P).transpose(3, 0, 2, 1)
            im[f"w2_{e}"] = np.ascontiguousarray(b.reshape(P, CO2 * KO2 * P)).astype(bf16)
        in_maps.append(im)

    import os

    trace = bool(int(os.environ.get("MOE_TRACE", "0")))
    if trace:
        _ensure_ntff_hook()

    global last_result
    res = run_bass_kernel_spmd(
        nc,
        in_maps,
        core_ids=list(range(N_EXPERTS)),
        trace=trace,
    )
    last_result = res

    ysum = np.zeros((C, TP), dtype=np.float32)
    for k in range(N_EXPERTS):
        yk = res.results[k]["ytp"].reshape(n_tiles, P, CO2, T_TILE)
        for ti, (e, t0, Tt) in enumerate(tiles[:-1]):
            blk = yk[ti, :, :, :Tt].transpose(1, 0, 2).reshape(C, Tt)
            ysum[:, t0 : t0 + Tt] += blk.astype(np.float32)
        e, t0, Tt = tiles[-1]
        blk = (
            res.results[k]["ytail"]
            .reshape(P, CO2, Tt)
            .transpose(1, 0, 2)
            .reshape(C, Tt)
        )
        ysum[:, t0 : t0 + Tt] += blk.astype(np.float32)
    contrib = (ysum * pair_w[None, :]).T  # [TP, C]
    out = np.zeros((T, C), dtype=np.float32)
    np.add.at(out, pair_tok, contrib)
    return out.reshape(B, N, C)
